# revision 38
# baseline (speedup 1.0000x reference)
import numpy as np

import concourse.bass as bass
import concourse.tile as tile
from concourse import bacc, mybir
from concourse.bass_utils import run_bass_kernel_spmd

F32 = mybir.dt.float32
F32R = mybir.dt.float32r
BF16 = mybir.dt.bfloat16

B, S, D = 4, 2048, 1024
H, HD = 16, 64
SH = S // 2
NCORES = 8
LPAIRS = 4
DC = D // 128
ST = S // 128
QB = S // 512
KC = S // 128
SCALE = 1.0 / np.sqrt(HD)
GROUPS = [[0, 1], [2, 3], [4, 5], [6, 7]]


def _pbcast1d(ap1d, parts):
    return bass.AP(tensor=ap1d.tensor, offset=ap1d.offset,
                   ap=[[0, parts]] + list(ap1d.ap))


BENCH_LOOP = 0
BENCH_NO_COLL = 0
QB_ORDER = (2, 0, 3, 1)
PROJ_LATE = 1
XSPLIT = 1
NORM_SPLIT = 1


def build_graph(nc, tc, ctx):
    if BENCH_LOOP:
        from contextlib import ExitStack
        with tc.For_i(0, BENCH_LOOP):
            with ExitStack() as inner:
                _graph_body(nc, tc, inner)
    else:
        _graph_body(nc, tc, ctx)


def _graph_body(nc, tc, ctx):
    xt_d = nc.dram_tensor("XT", [D, S], BF16, kind="ExternalInput")
    wqkv_d = nc.dram_tensor("Wqkv", [D, 1536], BF16, kind="ExternalInput")
    bqkt_d = nc.dram_tensor("bqkT", [128, 8], F32, kind="ExternalInput")
    bv_d = nc.dram_tensor("bv", [512], F32, kind="ExternalInput")
    wproj_d = nc.dram_tensor("Wproj", [D, D], BF16, kind="ExternalInput")
    bproj_d = nc.dram_tensor("bproj", [D], F32, kind="ExternalInput")
    out_d = nc.dram_tensor("out", [SH, D], F32, kind="ExternalOutput")

    const = ctx.enter_context(tc.tile_pool(name="const", bufs=1))
    stage = ctx.enter_context(tc.tile_pool(name="stage", bufs=2))
    xtp = ctx.enter_context(tc.tile_pool(name="xtp", bufs=1))
    wvp = ctx.enter_context(tc.tile_pool(name="wvp", bufs=1))
    wpp = ctx.enter_context(tc.tile_pool(name="wpp", bufs=1))
    wpairp = ctx.enter_context(tc.tile_pool(name="wpairp", bufs=2))
    qktp = ctx.enter_context(tc.tile_pool(name="qktp", bufs=2))
    vop = ctx.enter_context(tc.tile_pool(name="vop", bufs=1))
    ptp = ctx.enter_context(tc.tile_pool(name="ptp", bufs=8))
    otp = ctx.enter_context(tc.tile_pool(name="otp", bufs=1))
    rotp = ctx.enter_context(tc.tile_pool(name="rotp", bufs=1))
    recp = ctx.enter_context(tc.tile_pool(name="recp", bufs=2))
    outp = ctx.enter_context(tc.tile_pool(name="outp", bufs=2))
    dramp = ctx.enter_context(tc.tile_pool(name="dramp", bufs=1, space="DRAM"))
    psum = ctx.enter_context(tc.tile_pool(name="psum", bufs=1, space="PSUM"))

    def big_psum(name):
        return psum.tile([128, 1024], F32, tag="big", bufs=3, name=name)

    bq_cols = const.tile([128, 8], F32)
    nc.gpsimd.dma_start(out=bq_cols, in_=bqkt_d.ap())

    bv_bcast = const.tile([128, 512], F32)
    nc.gpsimd.dma_start(out=bv_bcast, in_=_pbcast1d(bv_d.ap(), 128))
    bp_bcast = const.tile([128, D], F32)
    nc.gpsimd.dma_start(out=bp_bcast, in_=_pbcast1d(bproj_d.ap(), 128))
    ones_f = const.tile([HD + 1, HD], F32)
    nc.vector.memset(ones_f, 1.0)
    ones_bc = const.tile([HD + 1, HD], F32R)
    nc.vector.tensor_copy(ones_bc, ones_f)

    wpair0 = []
    for dc in range(DC):
        wbf = stage.tile([128, 256], BF16, tag="wpst", bufs=8, name="wpst")
        t = wqkv_d.ap()
        src = bass.AP(tensor=t.tensor, offset=t.offset + 1536 * 128 * dc,
                      ap=[[1536, 128], [512, 2], [1, 128]])
        nc.gpsimd.dma_start(out=wbf.rearrange("p (c n) -> p c n", c=2),
                            in_=src)
        wpair0.append(wbf)
    xt_bf = []
    for dc in range(DC):
        xbf = xtp.tile([128, S], BF16, tag=f"xt{dc}", name=f"xt{dc}")
        for ha in range(2):
            eng = nc.scalar if (XSPLIT and ha == 1) else nc.sync
            eng.dma_start(
                out=xbf[:, SH * ha:SH * (ha + 1)],
                in_=xt_d.ap()[128 * dc:128 * (dc + 1), SH * ha:SH * (ha + 1)])
        xt_bf.append(xbf)
    wv_bf = []
    for dc in range(DC):
        wbf = wvp.tile([128, 512], BF16, tag=f"wv{dc}", bufs=1, name=f"wv{dc}")
        nc.sync.dma_start(out=wbf,
                          in_=wqkv_d.ap()[128 * dc:128 * (dc + 1), 1024:1536])
        wv_bf.append(wbf)

    vo = [None] * ST

    def emit_v(st):
        vps = big_psum(f"vps{st}")
        for dc in range(DC):
            nc.tensor.matmul(
                vps[:, 0:512],
                xt_bf[dc][:, 128 * st:128 * (st + 1)],
                wv_bf[dc],
                start=(dc == 0),
                stop=(dc == DC - 1),
            )
        vt = vop.tile([128, 8, HD + 1], BF16, tag=f"vo{st}", name=f"vo{st}")
        nc.vector.tensor_add(
            vt[:, :, 0:HD],
            vps[:, 0:512].rearrange("p (h e) -> p h e", h=8),
            bv_bcast.rearrange("p (h e) -> p h e", h=8),
        )
        nc.vector.memset(vt[:, :, HD:HD + 1], 1.0)
        vo[st] = vt

    def emit_wpair(hp):
        wpair = []
        for dc in range(DC):
            wbf = wpairp.tile([128, 256], BF16, tag=f"wqk{dc}", bufs=2,
                              name=f"wqk{dc}")
            nc.sync.dma_start(
                out=wbf[:, 0:128],
                in_=wqkv_d.ap()[128 * dc:128 * (dc + 1),
                                128 * hp:128 * (hp + 1)],
            )
            nc.sync.dma_start(
                out=wbf[:, 128:256],
                in_=wqkv_d.ap()[128 * dc:128 * (dc + 1),
                                512 + 128 * hp:512 + 128 * (hp + 1)],
            )
            wpair.append(wbf)
        return wpair

    def qk_pieces(hp, wpair, sink):
        qt = qktp.tile([128, S], BF16, tag="qt", bufs=2, name=f"qt{hp}")
        kt = qktp.tile([128, S], BF16, tag="kt", bufs=2, name=f"kt{hp}")
        sink["qt"], sink["kt"] = qt, kt
        pieces = []
        holder = {}

        def qk_mm(which, half, dc, nb):
            wslice = (slice(0, 128) if which == "q" else slice(128, 256))
            bias_col = hp if which == "q" else 4 + hp
            dst = qt if which == "q" else kt

            def f():
                key = f"{which}{half}"
                if key not in holder:
                    holder[key] = big_psum(f"{which}ps{hp}_{half}")
                nc.tensor.matmul(
                    holder[key][:, 512 * nb:512 * (nb + 1)],
                    wpair[dc][:, wslice],
                    xt_bf[dc][:, SH * half + 512 * nb:SH * half + 512 * (nb + 1)],
                    start=(dc == 0),
                    stop=(dc == DC - 1),
                )
                if dc == DC - 1:
                    nc.vector.tensor_scalar_add(
                        dst[:, SH * half + 512 * nb:SH * half + 512 * (nb + 1)],
                        holder[key][:, 512 * nb:512 * (nb + 1)],
                        bq_cols[:, bias_col:bias_col + 1])
            return f

        for which in ("q", "k"):
            for half in range(2):
                for dc in range(DC):
                    for nb in range(2):
                        pieces.append(qk_mm(which, half, dc, nb))
        return pieces

    pv_backlog = []

    def emit_pv(hp, pv, kc, pt):
        for h in range(2):
            nc.tensor.matmul(
                pv[h][0:HD + 1, :],
                vo[kc][:, 2 * hp + h, :],
                pt[:, 512 * h:512 * (h + 1)],
                start=(kc == 0),
                stop=(kc == KC - 1),
            )

    pending_norm = [None]
    pending_normb = [None]

    def flush_norm_a():
        if pending_norm[0] is None:
            return
        hp, qb, pv, ott = pending_norm[0]
        pending_norm[0] = None
        sums2 = []
        for h in range(2):
            sums = recp.tile([HD + 1, 512], F32R, tag=f"sums{h}", bufs=2,
                             name=f"sums{h}")
            nc.vector.tensor_copy(sums[HD:HD + 1, :], pv[h][HD:HD + 1, :])
            sums2.append(sums)
        pending_normb[0] = (hp, qb, pv, ott, sums2)

    def flush_norm_b():
        if pending_normb[0] is None:
            return
        hp, qb, pv, ott, sums2 = pending_normb[0]
        pending_normb[0] = None
        bc = big_psum(f"bc{hp}_{qb}")
        for h in range(2):
            nc.tensor.matmul(
                bc[0:HD, 512 * h:512 * (h + 1)],
                ones_bc[HD:HD + 1, :],
                sums2[h][HD:HD + 1, :],
                start=True,
                stop=True,
                tile_position=(64, 0),
            )
        for h in range(2):
            recb = recp.tile([64, 512], F32, tag=f"recb{h}", bufs=2,
                             name=f"recb{h}")
            nc.vector.reciprocal_approx_fast(
                recb, bc[0:HD, 512 * h:512 * (h + 1)])
            if h == 0:
                nc.vector.tensor_mul(
                    ott[0:64, 512 * qb:512 * (qb + 1)], pv[h][0:HD, :], recb
                )
            else:
                otmp = recp.tile([64, 512], BF16, tag="otmp", bufs=2,
                                 name="otmp")
                nc.vector.tensor_mul(otmp, pv[h][0:HD, :], recb)
                nc.sync.dma_start(
                    out=ott[64:128, 512 * qb:512 * (qb + 1)], in_=otmp
                )
        if qb >= 2:
            emit_xchg_half(hp, qb - 2, ott)

    def flush_norm():
        flush_norm_a()
        flush_norm_b()

    rot = [None] * LPAIRS

    def emit_xchg_half(hp, half, ott):
        lo = SH + 512 * half
        inb = dramp.tile([128, 512], BF16, name=f"inb{hp}_{half}")
        outb = dramp.tile([2, 128, 512], BF16, name=f"outb{hp}_{half}")
        nc.sync.dma_start(out=inb, in_=ott[:, lo:lo + 512])
        if BENCH_NO_COLL:
            nc.gpsimd.dma_start(out=outb[0], in_=inb)
            nc.gpsimd.dma_start(out=outb[1], in_=inb)
        else:
            nc.gpsimd.collective_compute(
                "AllGather",
                mybir.AluOpType.bypass,
                ins=[inb.opt()],
                outs=[outb.opt()],
                replica_groups=GROUPS,
            )
        both = rotp.tile([128, 2, 512], BF16, tag="both", bufs=2,
                         name=f"both{hp}_{half}")
        nc.sync.dma_start(out=both, in_=outb.rearrange("c p n -> p c n"))
        ssum = rotp.tile([128, 512], F32, tag="ssum", bufs=2,
                         name=f"ssum{hp}_{half}")
        nc.vector.tensor_add(ssum, both[:, 0, :], both[:, 1, :])
        if rot[hp] is None:
            rot[hp] = rotp.tile([128, SH], BF16, tag=f"rot{hp}",
                                name=f"rot{hp}")
        nc.vector.tensor_sub(rot[hp][:, 512 * half:512 * (half + 1)], ssum,
                             ott[:, lo:lo + 512])

    def attention_pair(hp, qt, kt, fillers, ott, late_windows=(), due=None):
        nfill = len(fillers)
        fi = 0
        lstate = [[0, s, e, pieces] for (s, e, pieces) in late_windows]
        for qbi, qb in enumerate(QB_ORDER):
            pv = [
                psum.tile([128, 512], F32, tag=f"pv{h}", bufs=1, name=f"pv{h}")
                for h in range(2)
            ]
            for kc in range(KC):
                step = qbi * KC + kc
                if due:
                    for p in due.pop(step, ()):
                        p()
                if hp == 0:
                    want = (0 if qbi == 0
                            else min(nfill,
                                     (step - KC + 4) * nfill // (3 * KC)))
                else:
                    want = min(nfill, (step + 4) * nfill // (QB * KC))
                while fi < want:
                    fillers[fi]()
                    fi += 1
                for st in lstate:
                    if step < st[1]:
                        continue
                    pieces = st[3]
                    lwant = min(len(pieces),
                                (step - st[1] + 1) * len(pieces)
                                // (st[2] - st[1] + 1))
                    while st[0] < lwant:
                        pieces[st[0]]()
                        st[0] += 1
                scps = big_psum(f"sc{hp}_{qb}_{kc}")
                for h in range(2):
                    nc.tensor.matmul(
                        scps[:, 512 * h:512 * (h + 1)],
                        kt[64 * h:64 * (h + 1), 128 * kc:128 * (kc + 1)],
                        qt[64 * h:64 * (h + 1), 512 * qb:512 * (qb + 1)],
                        start=True,
                        stop=True,
                    )
                pt = ptp.tile([128, 1024], BF16, tag="pt", bufs=8, name="pt")
                nc.scalar.activation(pt, scps, mybir.ActivationFunctionType.Exp,
                                     scale=SCALE)
                if hp == 0 and qbi == 0:
                    emit_v(kc)
                if kc == 0:
                    while pv_backlog:
                        pv_backlog.pop(0)()
                    flush_norm_a()
                    if not NORM_SPLIT:
                        flush_norm_b()
                elif kc == 1 and NORM_SPLIT:
                    pass
                elif kc == 2 and NORM_SPLIT:
                    flush_norm_b()
                else:
                    while pv_backlog:
                        pv_backlog.pop(0)()
                pv_backlog.append(
                    lambda hp=hp, pv=pv, kc=kc, pt=pt: emit_pv(hp, pv, kc, pt)
                )
            pending_norm[0] = (hp, qb, pv, ott)
        for st in lstate:
            while st[0] < len(st[3]):
                st[3][st[0]]()
                st[0] += 1

    sink = {}
    qt0 = qktp.tile([128, S], BF16, tag="qt", bufs=2, name="qt0a")
    kt0 = qktp.tile([128, S], BF16, tag="kt", bufs=2, name="kt0a")
    sink["qt"], sink["kt"] = qt0, kt0

    def qk0_group(which, half, nb, ps):
        wslice = slice(0, 128) if which == "q" else slice(128, 256)
        dst = qt0 if which == "q" else kt0
        bias_col = 0 if which == "q" else 4

        def mk(dc):
            def f():
                nc.tensor.matmul(
                    ps,
                    wpair0[dc][:, wslice],
                    xt_bf[dc][:, SH * half + 512 * nb:
                              SH * half + 512 * (nb + 1)],
                    start=(dc == 0),
                    stop=(dc == DC - 1),
                )
                if dc == DC - 1:
                    nc.vector.tensor_scalar_add(
                        dst[:, SH * half + 512 * nb:
                            SH * half + 512 * (nb + 1)],
                        ps, bq_cols[:, bias_col:bias_col + 1])
            return f
        return [mk(dc) for dc in range(DC)]

    due0 = None
    b1 = big_psum("qk0a")
    b2 = big_psum("qk0b")
    b3 = big_psum("qk0c")
    p7 = psum.tile([128, 512], F32, tag="pv0", bufs=1, name="qk0d")
    p8 = psum.tile([128, 512], F32, tag="pv1", bufs=1, name="qk0e")
    pre_groups = [
        qk0_group("q", 1, 0, b1[:, 0:512]),
        qk0_group("k", 0, 0, b1[:, 512:1024]),
        qk0_group("k", 0, 1, b2[:, 0:512]),
        qk0_group("k", 1, 0, b2[:, 512:1024]),
        qk0_group("k", 1, 1, b3[:, 0:512]),
        qk0_group("q", 0, 0, b3[:, 512:1024]),
        qk0_group("q", 1, 1, p7),
        qk0_group("q", 0, 1, p8),
    ]
    for dc in range(DC):
        for g in pre_groups:
            g[dc]()

    wproj_bf = []
    ot = []

    def proj_group(qi):
        holder = {}

        def mk(c):
            def f():
                if "pps" not in holder:
                    holder["pps"] = big_psum(f"pps{qi}")
                pps = holder["pps"]
                lhs = (ot[c][:, 128 * qi:128 * (qi + 1)] if c < LPAIRS
                       else rot[c - LPAIRS][:, 128 * qi:128 * (qi + 1)])
                for nb in range(2):
                    nc.tensor.matmul(
                        pps[:, 512 * nb:512 * (nb + 1)],
                        lhs,
                        wproj_bf[c][:, 512 * nb:512 * (nb + 1)],
                        start=(c == 0),
                        stop=(c == 7),
                    )
                if c == 7:
                    ost = outp.tile([128, D], F32, tag="ost", bufs=2,
                                    name="ost")
                    nc.vector.tensor_add(ost, pps, bp_bcast)
                    nc.sync.dma_start(
                        out=out_d.ap()[128 * qi:128 * (qi + 1), :], in_=ost)
            return f
        return [mk(c) for c in range(8)]

    for hp in range(LPAIRS):
        qt, kt = sink["qt"], sink["kt"]
        fillers = []
        late = []
        if hp < LPAIRS - 1:
            wpair_n = emit_wpair(hp + 1)
            sink = {}
            fillers = qk_pieces(hp + 1, wpair_n, sink)
        elif PROJ_LATE:
            late = [(2 * KC + 1, 4 * KC - 1,
                     [p for qi in range(4) for p in proj_group(qi)])]
        if hp == 0:
            for dc in range(DC):
                wbf = wpp.tile([128, D], BF16, tag=f"wp{dc}", bufs=1,
                               name=f"wp{dc}")
                nc.sync.dma_start(
                    out=wbf, in_=wproj_d.ap()[128 * dc:128 * (dc + 1), :])
                wproj_bf.append(wbf)
        ott = otp.tile([128, S], BF16, tag=f"ot{hp}", name=f"ot{hp}")
        ot.append(ott)
        attention_pair(hp, qt, kt, fillers, ott, late,
                       due0 if hp == 0 else None)

    while pv_backlog:
        pv_backlog.pop(0)()
    flush_norm()

    for qi in range((4 if PROJ_LATE else 0), SH // 128):
        for piece in proj_group(qi):
            piece()


def build_nc():
    from contextlib import ExitStack

    nc = bacc.Bacc("TRN2", target_bir_lowering=False, debug=False,
                   num_devices=NCORES)
    with tile.TileContext(nc) as tc:
        with ExitStack() as ctx:
            build_graph(nc, tc, ctx)
    nc.compile()
    return nc


def make_in_maps(X, W_qkv, b_qkv, W_proj, b_proj):
    import ml_dtypes
    bf16 = ml_dtypes.bfloat16
    X = np.asarray(X, dtype=np.float32).astype(bf16)
    wqkv = np.asarray(W_qkv, dtype=np.float32).astype(bf16)
    bqkv = np.asarray(b_qkv, dtype=np.float32)
    wproj = np.asarray(W_proj, dtype=np.float32).astype(bf16)
    bproj = np.ascontiguousarray(np.asarray(b_proj, dtype=np.float32))
    xts = [np.ascontiguousarray(X[b].T) for b in range(B)]
    in_maps = []
    for i in range(NCORES):
        b, hh = divmod(i, 2)
        xt = xts[b] if hh == 0 else np.ascontiguousarray(
            np.roll(xts[b], -SH, axis=1))
        o = 512 * hh
        wq = wqkv[:, o:o + 512]
        wk = wqkv[:, D + o:D + o + 512]
        wv = wqkv[:, 2 * D + o:2 * D + o + 512]
        w_core = np.ascontiguousarray(np.concatenate([wq, wk, wv], axis=1))
        bqk = np.concatenate([bqkv[o:o + 512], bqkv[D + o:D + o + 512]])
        bqkt = np.ascontiguousarray(bqk.reshape(8, 128).T)
        bv = np.ascontiguousarray(bqkv[2 * D + o:2 * D + o + 512])
        wp_core = np.ascontiguousarray(np.concatenate(
            [wproj[o:o + 512], wproj[512 * (1 - hh):512 * (1 - hh) + 512]],
            axis=0))
        in_maps.append({
            "XT": xt, "Wqkv": w_core, "bqkT": bqkt, "bv": bv,
            "Wproj": wp_core, "bproj": bproj,
        })
    return in_maps


_NC_CACHE = None


def get_nc():
    global _NC_CACHE
    if _NC_CACHE is None:
        _NC_CACHE = build_nc()
    return _NC_CACHE


def kernel(X, W_qkv, b_qkv, W_proj, b_proj):
    nc = get_nc()
    in_maps = make_in_maps(X, W_qkv, b_qkv, W_proj, b_proj)
    res = run_bass_kernel_spmd(nc, in_maps, core_ids=list(range(NCORES)))
    out = np.empty((B, S, D), np.float32)
    for i in range(NCORES):
        b, hh = divmod(i, 2)
        out[b, hh * SH:(hh + 1) * SH] = res.results[i]["out"]
    return out


# revision 40
# speedup vs baseline: 1.1875x; 1.1875x over previous
import numpy as np

import concourse.bass as bass
import concourse.tile as tile
from concourse import bacc, mybir
from concourse.bass_utils import run_bass_kernel_spmd

F32 = mybir.dt.float32
F32R = mybir.dt.float32r
BF16 = mybir.dt.bfloat16

B, S, D = 4, 2048, 1024
H, HD = 16, 64
SH = S // 2
NCORES = 8
LPAIRS = 4
DC = D // 128
ST = S // 128
QB = S // 512
KC = S // 128
SCALE = 1.0 / np.sqrt(HD)
GROUPS = [[0, 1], [2, 3], [4, 5], [6, 7]]


def _pbcast1d(ap1d, parts):
    return bass.AP(tensor=ap1d.tensor, offset=ap1d.offset,
                   ap=[[0, parts]] + list(ap1d.ap))


BENCH_LOOP = 0
BENCH_NO_COLL = 0
QB_ORDER = (2, 0, 3, 1)
PROJ_LATE = 1
XSPLIT = 1
NORM_SPLIT = 1


def build_graph(nc, tc, ctx):
    if BENCH_LOOP:
        from contextlib import ExitStack
        with tc.For_i(0, BENCH_LOOP):
            with ExitStack() as inner:
                _graph_body(nc, tc, inner)
    else:
        _graph_body(nc, tc, ctx)


def _graph_body(nc, tc, ctx):
    xt_d = nc.dram_tensor("XT", [D, S], BF16, kind="ExternalInput")
    wqkv_d = nc.dram_tensor("Wqkv", [D, 1536], BF16, kind="ExternalInput")
    bqkt_d = nc.dram_tensor("bqkT", [128, 8], F32, kind="ExternalInput")
    bv_d = nc.dram_tensor("bv", [512], F32, kind="ExternalInput")
    wproj_d = nc.dram_tensor("Wproj", [D, D], BF16, kind="ExternalInput")
    bproj_d = nc.dram_tensor("bproj", [D], F32, kind="ExternalInput")
    out_d = nc.dram_tensor("out", [SH, D], F32, kind="ExternalOutput")

    const = ctx.enter_context(tc.tile_pool(name="const", bufs=1))
    stage = ctx.enter_context(tc.tile_pool(name="stage", bufs=2))
    xtp = ctx.enter_context(tc.tile_pool(name="xtp", bufs=1))
    wvp = ctx.enter_context(tc.tile_pool(name="wvp", bufs=1))
    wpp = ctx.enter_context(tc.tile_pool(name="wpp", bufs=1))
    wpairp = ctx.enter_context(tc.tile_pool(name="wpairp", bufs=2))
    qktp = ctx.enter_context(tc.tile_pool(name="qktp", bufs=2))
    vop = ctx.enter_context(tc.tile_pool(name="vop", bufs=1))
    ptp = ctx.enter_context(tc.tile_pool(name="ptp", bufs=8))
    otp = ctx.enter_context(tc.tile_pool(name="otp", bufs=1))
    rotp = ctx.enter_context(tc.tile_pool(name="rotp", bufs=1))
    recp = ctx.enter_context(tc.tile_pool(name="recp", bufs=2))
    outp = ctx.enter_context(tc.tile_pool(name="outp", bufs=2))
    dramp = ctx.enter_context(tc.tile_pool(name="dramp", bufs=1, space="DRAM"))
    psum = ctx.enter_context(tc.tile_pool(name="psum", bufs=1, space="PSUM"))

    def big_psum(name):
        return psum.tile([128, 1024], F32, tag="big", bufs=3, name=name)

    bq_cols = const.tile([128, 8], F32)
    nc.gpsimd.dma_start(out=bq_cols, in_=bqkt_d.ap())

    bv_bcast = const.tile([128, 512], F32)
    nc.gpsimd.dma_start(out=bv_bcast, in_=_pbcast1d(bv_d.ap(), 128))
    bp_bcast = const.tile([128, D], F32)
    nc.gpsimd.dma_start(out=bp_bcast, in_=_pbcast1d(bproj_d.ap(), 128))
    ones_f = const.tile([HD + 1, HD], F32)
    nc.vector.memset(ones_f, 1.0)
    ones_bc = const.tile([HD + 1, HD], F32R)
    nc.vector.tensor_copy(ones_bc, ones_f)

    wpair0 = []
    xt_bf = []
    for dc in range(DC):
        xbf = xtp.tile([128, S], BF16, tag=f"xt{dc}", name=f"xt{dc}")
        for ha in range(2):
            eng = nc.scalar if (XSPLIT and ha == 1) else nc.sync
            eng.dma_start(
                out=xbf[:, SH * ha:SH * (ha + 1)],
                in_=xt_d.ap()[128 * dc:128 * (dc + 1), SH * ha:SH * (ha + 1)])
        xt_bf.append(xbf)
        wbf = stage.tile([128, 256], BF16, tag="wpst", bufs=8, name="wpst")
        nc.sync.dma_start(out=wbf[:, 0:128],
                          in_=wqkv_d.ap()[128 * dc:128 * (dc + 1), 0:128])
        nc.sync.dma_start(out=wbf[:, 128:256],
                          in_=wqkv_d.ap()[128 * dc:128 * (dc + 1), 512:640])
        wpair0.append(wbf)
    wv_bf = []
    for dc in range(DC):
        wbf = wvp.tile([128, 512], BF16, tag=f"wv{dc}", bufs=1, name=f"wv{dc}")
        nc.sync.dma_start(out=wbf,
                          in_=wqkv_d.ap()[128 * dc:128 * (dc + 1), 1024:1536])
        wv_bf.append(wbf)

    vo = [None] * ST

    def emit_v(st):
        vps = big_psum(f"vps{st}")
        for dc in range(DC):
            nc.tensor.matmul(
                vps[:, 0:512],
                xt_bf[dc][:, 128 * st:128 * (st + 1)],
                wv_bf[dc],
                start=(dc == 0),
                stop=(dc == DC - 1),
            )
        vt = vop.tile([128, 8, HD + 1], BF16, tag=f"vo{st}", name=f"vo{st}")
        nc.vector.tensor_add(
            vt[:, :, 0:HD],
            vps[:, 0:512].rearrange("p (h e) -> p h e", h=8),
            bv_bcast.rearrange("p (h e) -> p h e", h=8),
        )
        nc.vector.memset(vt[:, :, HD:HD + 1], 1.0)
        vo[st] = vt

    def emit_wpair(hp):
        wpair = []
        for dc in range(DC):
            wbf = wpairp.tile([128, 256], BF16, tag=f"wqk{dc}", bufs=2,
                              name=f"wqk{dc}")
            nc.sync.dma_start(
                out=wbf[:, 0:128],
                in_=wqkv_d.ap()[128 * dc:128 * (dc + 1),
                                128 * hp:128 * (hp + 1)],
            )
            nc.sync.dma_start(
                out=wbf[:, 128:256],
                in_=wqkv_d.ap()[128 * dc:128 * (dc + 1),
                                512 + 128 * hp:512 + 128 * (hp + 1)],
            )
            wpair.append(wbf)
        return wpair

    def qk_pieces(hp, wpair, sink):
        qt = qktp.tile([128, S], BF16, tag="qt", bufs=2, name=f"qt{hp}")
        kt = qktp.tile([128, S], BF16, tag="kt", bufs=2, name=f"kt{hp}")
        sink["qt"], sink["kt"] = qt, kt
        pieces = []
        holder = {}

        def qk_mm(which, half, dc, nb):
            wslice = (slice(0, 128) if which == "q" else slice(128, 256))
            bias_col = hp if which == "q" else 4 + hp
            dst = qt if which == "q" else kt

            def f():
                key = f"{which}{half}"
                if key not in holder:
                    holder[key] = big_psum(f"{which}ps{hp}_{half}")
                nc.tensor.matmul(
                    holder[key][:, 512 * nb:512 * (nb + 1)],
                    wpair[dc][:, wslice],
                    xt_bf[dc][:, SH * half + 512 * nb:SH * half + 512 * (nb + 1)],
                    start=(dc == 0),
                    stop=(dc == DC - 1),
                )
                if dc == DC - 1:
                    nc.vector.tensor_scalar_add(
                        dst[:, SH * half + 512 * nb:SH * half + 512 * (nb + 1)],
                        holder[key][:, 512 * nb:512 * (nb + 1)],
                        bq_cols[:, bias_col:bias_col + 1])
            return f

        for which in ("q", "k"):
            for half in range(2):
                for dc in range(DC):
                    for nb in range(2):
                        pieces.append(qk_mm(which, half, dc, nb))
        return pieces

    pv_backlog = []

    def emit_pv(hp, pv, kc, pt):
        for h in range(2):
            nc.tensor.matmul(
                pv[h][0:HD + 1, :],
                vo[kc][:, 2 * hp + h, :],
                pt[:, 512 * h:512 * (h + 1)],
                start=(kc == 0),
                stop=(kc == KC - 1),
            )

    pending_norm = [None]
    pending_normb = [None]

    def flush_norm_a():
        if pending_norm[0] is None:
            return
        hp, qb, pv, ott = pending_norm[0]
        pending_norm[0] = None
        sums2 = []
        for h in range(2):
            sums = recp.tile([HD + 1, 512], F32R, tag=f"sums{h}", bufs=2,
                             name=f"sums{h}")
            nc.vector.tensor_copy(sums[HD:HD + 1, :], pv[h][HD:HD + 1, :])
            sums2.append(sums)
        pending_normb[0] = (hp, qb, pv, ott, sums2)

    def flush_norm_b():
        if pending_normb[0] is None:
            return
        hp, qb, pv, ott, sums2 = pending_normb[0]
        pending_normb[0] = None
        bc = big_psum(f"bc{hp}_{qb}")
        for h in range(2):
            nc.tensor.matmul(
                bc[0:HD, 512 * h:512 * (h + 1)],
                ones_bc[HD:HD + 1, :],
                sums2[h][HD:HD + 1, :],
                start=True,
                stop=True,
                tile_position=(64, 0),
            )
        for h in range(2):
            recb = recp.tile([64, 512], F32, tag=f"recb{h}", bufs=2,
                             name=f"recb{h}")
            nc.vector.reciprocal_approx_fast(
                recb, bc[0:HD, 512 * h:512 * (h + 1)])
            if h == 0:
                nc.vector.tensor_mul(
                    ott[0:64, 512 * qb:512 * (qb + 1)], pv[h][0:HD, :], recb
                )
            else:
                otmp = recp.tile([64, 512], BF16, tag="otmp", bufs=2,
                                 name="otmp")
                nc.vector.tensor_mul(otmp, pv[h][0:HD, :], recb)
                nc.sync.dma_start(
                    out=ott[64:128, 512 * qb:512 * (qb + 1)], in_=otmp
                )
        if qb >= 2:
            emit_xchg_half(hp, qb - 2, ott)

    def flush_norm():
        flush_norm_a()
        flush_norm_b()

    rot = [None] * LPAIRS

    def emit_xchg_half(hp, half, ott):
        lo = SH + 512 * half
        inb = dramp.tile([128, 512], BF16, name=f"inb{hp}_{half}")
        outb = dramp.tile([2, 128, 512], BF16, name=f"outb{hp}_{half}")
        nc.sync.dma_start(out=inb, in_=ott[:, lo:lo + 512])
        if BENCH_NO_COLL:
            nc.gpsimd.dma_start(out=outb[0], in_=inb)
            nc.gpsimd.dma_start(out=outb[1], in_=inb)
        else:
            nc.gpsimd.collective_compute(
                "AllGather",
                mybir.AluOpType.bypass,
                ins=[inb.opt()],
                outs=[outb.opt()],
                replica_groups=GROUPS,
            )
        both = rotp.tile([128, 2, 512], BF16, tag="both", bufs=2,
                         name=f"both{hp}_{half}")
        nc.sync.dma_start(out=both, in_=outb.rearrange("c p n -> p c n"))
        ssum = rotp.tile([128, 512], F32, tag="ssum", bufs=2,
                         name=f"ssum{hp}_{half}")
        nc.vector.tensor_add(ssum, both[:, 0, :], both[:, 1, :])
        if rot[hp] is None:
            rot[hp] = rotp.tile([128, SH], BF16, tag=f"rot{hp}",
                                name=f"rot{hp}")
        nc.vector.tensor_sub(rot[hp][:, 512 * half:512 * (half + 1)], ssum,
                             ott[:, lo:lo + 512])

    def attention_pair(hp, qt, kt, fillers, ott, late_windows=(), due=None):
        nfill = len(fillers)
        fi = 0
        lstate = [[0, s, e, pieces] for (s, e, pieces) in late_windows]
        for qbi, qb in enumerate(QB_ORDER):
            pv = [
                psum.tile([128, 512], F32, tag=f"pv{h}", bufs=1, name=f"pv{h}")
                for h in range(2)
            ]
            for kc in range(KC):
                step = qbi * KC + kc
                if due:
                    for p in due.pop(step, ()):
                        p()
                if hp == 0:
                    want = (0 if qbi == 0
                            else min(nfill,
                                     (step - KC + 4) * nfill // (3 * KC)))
                else:
                    want = min(nfill, (step + 4) * nfill // (QB * KC))
                while fi < want:
                    fillers[fi]()
                    fi += 1
                for st in lstate:
                    if step < st[1]:
                        continue
                    pieces = st[3]
                    lwant = min(len(pieces),
                                (step - st[1] + 1) * len(pieces)
                                // (st[2] - st[1] + 1))
                    while st[0] < lwant:
                        pieces[st[0]]()
                        st[0] += 1
                scps = big_psum(f"sc{hp}_{qb}_{kc}")
                for h in range(2):
                    nc.tensor.matmul(
                        scps[:, 512 * h:512 * (h + 1)],
                        kt[64 * h:64 * (h + 1), 128 * kc:128 * (kc + 1)],
                        qt[64 * h:64 * (h + 1), 512 * qb:512 * (qb + 1)],
                        start=True,
                        stop=True,
                    )
                pt = ptp.tile([128, 1024], BF16, tag="pt", bufs=8, name="pt")
                nc.scalar.activation(pt, scps, mybir.ActivationFunctionType.Exp,
                                     scale=SCALE)
                if hp == 0 and qbi == 0:
                    emit_v(kc)
                if kc == 0:
                    while pv_backlog:
                        pv_backlog.pop(0)()
                    flush_norm_a()
                    if not NORM_SPLIT:
                        flush_norm_b()
                elif kc == 1 and NORM_SPLIT:
                    pass
                elif kc == 2 and NORM_SPLIT:
                    flush_norm_b()
                else:
                    while pv_backlog:
                        pv_backlog.pop(0)()
                pv_backlog.append(
                    lambda hp=hp, pv=pv, kc=kc, pt=pt: emit_pv(hp, pv, kc, pt)
                )
            pending_norm[0] = (hp, qb, pv, ott)
        for st in lstate:
            while st[0] < len(st[3]):
                st[3][st[0]]()
                st[0] += 1

    due0 = None
    sink = {}
    for piece in qk_pieces(0, wpair0, sink):
        piece()

    wproj_bf = []
    ot = []

    def proj_group(qi):
        holder = {}

        def mk(c):
            def f():
                if "pps" not in holder:
                    holder["pps"] = big_psum(f"pps{qi}")
                pps = holder["pps"]
                lhs = (ot[c][:, 128 * qi:128 * (qi + 1)] if c < LPAIRS
                       else rot[c - LPAIRS][:, 128 * qi:128 * (qi + 1)])
                for nb in range(2):
                    nc.tensor.matmul(
                        pps[:, 512 * nb:512 * (nb + 1)],
                        lhs,
                        wproj_bf[c][:, 512 * nb:512 * (nb + 1)],
                        start=(c == 0),
                        stop=(c == 7),
                    )
                if c == 7:
                    ost = outp.tile([128, D], F32, tag="ost", bufs=2,
                                    name="ost")
                    nc.vector.tensor_add(ost, pps, bp_bcast)
                    nc.sync.dma_start(
                        out=out_d.ap()[128 * qi:128 * (qi + 1), :], in_=ost)
            return f
        return [mk(c) for c in range(8)]

    for hp in range(LPAIRS):
        qt, kt = sink["qt"], sink["kt"]
        fillers = []
        late = []
        if hp < LPAIRS - 1:
            wpair_n = emit_wpair(hp + 1)
            sink = {}
            fillers = qk_pieces(hp + 1, wpair_n, sink)
        elif PROJ_LATE:
            late = [(2 * KC + 1, 4 * KC - 1,
                     [p for qi in range(4) for p in proj_group(qi)])]
        if hp == 0:
            for dc in range(DC):
                wbf = wpp.tile([128, D], BF16, tag=f"wp{dc}", bufs=1,
                               name=f"wp{dc}")
                nc.sync.dma_start(
                    out=wbf, in_=wproj_d.ap()[128 * dc:128 * (dc + 1), :])
                wproj_bf.append(wbf)
        ott = otp.tile([128, S], BF16, tag=f"ot{hp}", name=f"ot{hp}")
        ot.append(ott)
        attention_pair(hp, qt, kt, fillers, ott, late,
                       due0 if hp == 0 else None)

    while pv_backlog:
        pv_backlog.pop(0)()
    flush_norm()

    for qi in range((4 if PROJ_LATE else 0), SH // 128):
        for piece in proj_group(qi):
            piece()


def build_nc():
    from contextlib import ExitStack

    nc = bacc.Bacc("TRN2", target_bir_lowering=False, debug=False,
                   num_devices=NCORES)
    with tile.TileContext(nc) as tc:
        with ExitStack() as ctx:
            build_graph(nc, tc, ctx)
    nc.compile()
    return nc


def make_in_maps(X, W_qkv, b_qkv, W_proj, b_proj):
    import ml_dtypes
    bf16 = ml_dtypes.bfloat16
    X = np.asarray(X, dtype=np.float32).astype(bf16)
    wqkv = np.asarray(W_qkv, dtype=np.float32).astype(bf16)
    bqkv = np.asarray(b_qkv, dtype=np.float32)
    wproj = np.asarray(W_proj, dtype=np.float32).astype(bf16)
    bproj = np.ascontiguousarray(np.asarray(b_proj, dtype=np.float32))
    xts = [np.ascontiguousarray(X[b].T) for b in range(B)]
    in_maps = []
    for i in range(NCORES):
        b, hh = divmod(i, 2)
        xt = xts[b] if hh == 0 else np.ascontiguousarray(
            np.roll(xts[b], -SH, axis=1))
        o = 512 * hh
        wq = wqkv[:, o:o + 512]
        wk = wqkv[:, D + o:D + o + 512]
        wv = wqkv[:, 2 * D + o:2 * D + o + 512]
        w_core = np.ascontiguousarray(np.concatenate([wq, wk, wv], axis=1))
        bqk = np.concatenate([bqkv[o:o + 512], bqkv[D + o:D + o + 512]])
        bqkt = np.ascontiguousarray(bqk.reshape(8, 128).T)
        bv = np.ascontiguousarray(bqkv[2 * D + o:2 * D + o + 512])
        wp_core = np.ascontiguousarray(np.concatenate(
            [wproj[o:o + 512], wproj[512 * (1 - hh):512 * (1 - hh) + 512]],
            axis=0))
        in_maps.append({
            "XT": xt, "Wqkv": w_core, "bqkT": bqkt, "bv": bv,
            "Wproj": wp_core, "bproj": bproj,
        })
    return in_maps


_NC_CACHE = None


def get_nc():
    global _NC_CACHE
    if _NC_CACHE is None:
        _NC_CACHE = build_nc()
    return _NC_CACHE


def kernel(X, W_qkv, b_qkv, W_proj, b_proj):
    nc = get_nc()
    in_maps = make_in_maps(X, W_qkv, b_qkv, W_proj, b_proj)
    res = run_bass_kernel_spmd(nc, in_maps, core_ids=list(range(NCORES)))
    out = np.empty((B, S, D), np.float32)
    for i in range(NCORES):
        b, hh = divmod(i, 2)
        out[b, hh * SH:(hh + 1) * SH] = res.results[i]["out"]
    return out
